# revision 4
# baseline (speedup 1.0000x reference)
"""Graph Wavelet NN (2-layer) Trainium2 kernel, 8-core row-parallel, v5.

v4 + local head-start: the wavelet matrices are host-permuted per core so the
core's OWN contraction chunks sit at fixed program positions, consumable from
SBUF staging while the AllGather of the remote chunks is still in flight.
Remote gather chunks are loaded with partition-id-derived cyclic offsets
((pid+1+s) & 7) via SWDGE dynamic DMA, so the program stays SPMD-uniform.

Math per layer: out = (wavelets * f) @ (wavelets_inv @ (x @ W)); the filter is
folded into a row-scale of s, so ONE bf16 copy of wavelets[rows].T serves both
layers (SBUF-resident).
"""

import sys

if "/opt/trn_rl_repo" not in sys.path:
    sys.path.insert(0, "/opt/trn_rl_repo")

import numpy as np
import ml_dtypes

import concourse.bass as bass
import concourse.mybir as mybir
import concourse.tile as tile
from concourse import bacc, bass_utils

N = 8192
F = 512
C = 256
NCORES = 8
R = N // NCORES          # 1024 rows per core

F32 = mybir.dt.float32
BF16 = mybir.dt.bfloat16
NP_BF16 = ml_dtypes.bfloat16

NKC = N // 128           # 64 contraction chunks (program positions)
WVG = 4                  # kc chunks per 512-row stream tile
NWV = 16                 # stream tiles per layer
NMT = R // 128           # 8 local row tiles
HR = R // 2              # half row-block (512)
NREM = NCORES - 1        # 7 remote ranks


def build_kernel(sim_single_core=False):
    nc = bacc.Bacc(
        "TRN2",
        target_bir_lowering=False,
        debug=False,
        num_devices=1 if sim_single_core else NCORES,
    )

    xT = nc.dram_tensor("xT", [F, R], BF16, kind="ExternalInput")
    w1 = nc.dram_tensor("w1", [F, C], BF16, kind="ExternalInput")
    w2 = nc.dram_tensor("w2", [C, C], BF16, kind="ExternalInput")
    winvT = nc.dram_tensor("winvT", [N, R], BF16, kind="ExternalInput")
    wT = nc.dram_tensor("wT", [N, R], BF16, kind="ExternalInput")
    f1 = nc.dram_tensor("f1", [R], F32, kind="ExternalInput")
    f2 = nc.dram_tensor("f2", [R], F32, kind="ExternalInput")
    outT = nc.dram_tensor("outT", [C, R], F32, kind="ExternalOutput")

    rg = [list(range(NCORES))]

    with tile.TileContext(nc) as tc:
        with (
            tc.tile_pool(name="dram", bufs=1, space="DRAM") as dram,
            tc.tile_pool(name="const", bufs=1) as const,
            tc.tile_pool(name="winvp", bufs=4) as winvp,
            tc.tile_pool(name="tsp", bufs=6) as tsp,
            tc.tile_pool(name="stage", bufs=3) as stage,
            tc.tile_pool(name="outst", bufs=1) as outstp,
            tc.tile_pool(name="psA", bufs=2, space="PSUM") as psA,
            tc.tile_pool(name="psB", bufs=2, space="PSUM") as psB,
        ):
            halves = {}
            gath = {}
            for nm in ("t1", "s1", "t2", "s2"):
                halves[nm] = [
                    dram.tile([HR, C], BF16, name=f"{nm}{h}_d") for h in ("A", "B")
                ]
                gath[nm] = [
                    dram.tile(
                        [NCORES * HR, C], BF16,
                        addr_space="Local" if sim_single_core else "Shared",
                        name=f"{nm}{h}g_d",
                    )
                    for h in ("A", "B")
                ]

            # ---- persistent SBUF ----
            wT_sb = const.tile([128, NKC, R], BF16)        # 128KB/part, resident
            xT_sb = const.tile([128, F // 128, R], BF16)   # 8KB/part
            w1_sb = const.tile([128, F // 128, C], BF16)   # 2KB/part
            w2_sb = const.tile([128, C // 128, C], BF16)   # 1KB/part
            h1T_sb = const.tile([128, C // 128, R], BF16)  # 4KB/part
            f1_sb = const.tile([128, NMT], F32)
            f2_sb = const.tile([128, NMT], F32)

            for q in range(2):
                nc.sync.dma_start(
                    out=xT_sb[:, q * 2:(q + 1) * 2, :],
                    in_=xT.ap()[q * 256:(q + 1) * 256, :].rearrange(
                        "(kc p) m -> p kc m", p=128
                    ),
                )
            nc.sync.dma_start(
                out=w1_sb[:], in_=w1.ap().rearrange("(kc p) n -> p kc n", p=128)
            )
            nc.sync.dma_start(
                out=w2_sb[:], in_=w2.ap().rearrange("(kc p) n -> p kc n", p=128)
            )
            nc.sync.dma_start(
                out=f1_sb[:], in_=f1.ap().rearrange("(mt p) -> p mt", p=128)
            )
            nc.sync.dma_start(
                out=f2_sb[:], in_=f2.ap().rearrange("(mt p) -> p mt", p=128)
            )

            def all_gather(in_d, out_d):
                if sim_single_core:
                    for rr in range(NCORES):
                        nc.sync.dma_start(
                            out=out_d[rr * HR:(rr + 1) * HR, :], in_=in_d[:, :]
                        )
                else:
                    nc.gpsimd.collective_compute(
                        "AllGather",
                        mybir.AluOpType.bypass,
                        replica_groups=rg,
                        ins=[in_d.opt()],
                        outs=[out_d.opt()],
                    )

            # cyclic remote-rank offsets; host permutes wavelet rows to match
            pid = nc.scalar.partition_id()
            rrs = [(pid + (1 + s)) & 7 for s in range(NREM)]

            # wT program-position chunk-groups are contiguous rows of the
            # host-permuted tensor: group g = positions 4g..4g+3.
            def wt_load(g, eng=None):
                (eng or nc.sync).dma_start(
                    out=wT_sb[:, g * WVG:(g + 1) * WVG, :],
                    in_=wT.ap()[g * 512:(g + 1) * 512, :].rearrange(
                        "(kc p) m -> p kc m", p=128
                    ),
                )

            # ---- t = x_loc @ W, halves staged + stored + gathered ----
            def t_phase(lhsT_sb, nkc, w_sb, t_ds, name):
                sts = []
                for h in range(2):
                    st = stage.tile([128, 4, C], BF16, tag="st", name=f"st_{name}{h}")
                    sts.append(st)
                    for m4 in range(4):
                        mt = h * 4 + m4
                        pt = psA.tile(
                            [128, 2, 2, C], F32, tag="psA", name=f"pt_{name}{mt}"
                        )
                        for kc in range(nkc):
                            nc.tensor.matmul(
                                pt[:, 0, 0, :],
                                lhsT_sb[:, kc, mt * 128:(mt + 1) * 128],
                                w_sb[:, kc, :],
                                start=(kc == 0),
                                stop=(kc == nkc - 1),
                            )
                        nc.vector.tensor_copy(st[:, m4, :], pt[:, 0, 0, :])
                    nc.sync.dma_start(
                        out=t_ds[h][:, :].rearrange("(mt p) n -> p mt n", p=128),
                        in_=st[:],
                    )
                return sts

            # ---- s_loc = Winv[rows,:] @ t_full (permuted contraction order:
            # local rows first from staging, then remote half-A, then half-B),
            # scaled by filter[rows] ----
            def s_phase(t_sts, tg_ds, f_sb, s_ds, name):
                ps = [
                    psA.tile([128, 2, 2, C], F32, tag="psA", name=f"ps_{name}{i}")
                    for i in range(2)
                ]
                # rhs chunk supplier per stream tile index (16 total)
                rhs_of = {}
                rhs_of[0] = lambda k4: t_sts[0][:, k4, :]
                rhs_of[1] = lambda k4: t_sts[1][:, k4, :]
                for h in range(2):
                    for s in range(NREM):
                        tch = tsp.tile(
                            [128, 4, C], BF16, tag="ts", name=f"tch_{name}{h}{s}"
                        )
                        nc.scalar.dma_start(
                            out=tch[:],
                            in_=tg_ds[h][bass.ts(rrs[s], HR), :].rearrange(
                                "(kc p) n -> p kc n", p=128
                            ),
                        )
                        rhs_of[2 + h * NREM + s] = (
                            lambda k4, _t=tch: _t[:, k4, :]
                        )
                for wv_i in range(NWV):
                    wv = winvp.tile(
                        [128, WVG, R], BF16, tag="wv", name=f"wv_{name}{wv_i}"
                    )
                    nc.sync.dma_start(
                        out=wv[:],
                        in_=winvT.ap()[wv_i * 512:(wv_i + 1) * 512, :].rearrange(
                            "(kc p) m -> p kc m", p=128
                        ),
                    )
                    for k4 in range(WVG):
                        for mt in range(NMT):
                            # two 256-col accumulators share each PSUM bank;
                            # only the first of each pair may set start=True
                            # (start clears the whole bank)
                            nc.tensor.matmul(
                                ps[mt // 4][:, (mt % 4) // 2, (mt % 4) % 2, :],
                                wv[:, k4, mt * 128:(mt + 1) * 128],
                                rhs_of[wv_i](k4),
                                start=(wv_i == 0 and k4 == 0 and mt % 2 == 0),
                                stop=(wv_i == NWV - 1 and k4 == WVG - 1),
                                skip_group_check=True,
                            )
                sts = []
                for h in range(2):
                    st = stage.tile([128, 4, C], BF16, tag="st", name=f"sst_{name}{h}")
                    sts.append(st)
                    for m4 in range(4):
                        mt = h * 4 + m4
                        nc.vector.tensor_scalar_mul(
                            st[:, m4, :],
                            ps[mt // 4][:, (mt % 4) // 2, (mt % 4) % 2, :],
                            f_sb[:, mt:mt + 1],
                        )
                    nc.sync.dma_start(
                        out=s_ds[h][:, :].rearrange("(mt p) n -> p mt n", p=128),
                        in_=st[:],
                    )
                return sts

            # ---- outT = (wavelets[rows] @ s_full).T, local positions first ----
            def out_phase(s_sts, sg_ds, drain_cb, name, load_wt):
                po = [
                    psB.tile([128, R], F32, tag="psB", name=f"po_{name}{i}")
                    for i in range(2)
                ]
                n_kc = 0

                def mm4(pos, lhsT_of):
                    nonlocal n_kc
                    for k4 in range(WVG):
                        kc = pos + k4
                        for ch in range(2):
                            for mh in range(2):
                                nc.tensor.matmul(
                                    po[ch][:, mh * 512:(mh + 1) * 512],
                                    lhsT_of(k4, ch),
                                    wT_sb[:, kc, mh * 512:(mh + 1) * 512],
                                    start=(n_kc == 0),
                                    stop=(n_kc == NKC - 1),
                                    skip_group_check=True,
                                )
                        n_kc += 1

                # local head-start: positions 56..63 from s staging
                for h in range(2):
                    st = s_sts[h]
                    mm4(
                        56 + h * 4,
                        lambda k4, ch, _st=st: _st[:, k4, ch * 128:(ch + 1) * 128],
                    )
                # remote sweep: positions h*28 + s*4
                wt_g = 4
                for h in range(2):
                    for s in range(NREM):
                        sch = tsp.tile(
                            [128, 4, C], BF16, tag="ts", name=f"sch_{name}{h}{s}"
                        )
                        nc.scalar.dma_start(
                            out=sch[:],
                            in_=sg_ds[h][bass.ts(rrs[s], HR), :].rearrange(
                                "(kc p) n -> p kc n", p=128
                            ),
                        )
                        if load_wt:
                            for _ in range(2):
                                if wt_g < 14:
                                    wt_load(wt_g)
                                    wt_g += 1
                        mm4(
                            h * 28 + s * 4,
                            lambda k4, ch, _t=sch: _t[:, k4, ch * 128:(ch + 1) * 128],
                        )
                for ch in range(2):
                    drain_cb(ch, po[ch])

            # ================= layer 1 =================
            # upfront wT: local positions (56-63) + first remote groups (0-7);
            # on the scalar queue so they don't delay the winv stream on SP
            for g in (14, 15, 0, 1):
                wt_load(g, eng=nc.scalar)

            t1_sts = t_phase(xT_sb, F // 128, w1_sb, halves["t1"], "t1")
            all_gather(halves["t1"][0], gath["t1"][0])
            all_gather(halves["t1"][1], gath["t1"][1])
            s1_sts = s_phase(t1_sts, gath["t1"], f1_sb, halves["s1"], "s1")
            wt_load(2)
            wt_load(3)
            all_gather(halves["s1"][0], gath["s1"][0])
            all_gather(halves["s1"][1], gath["s1"][1])

            def relu_drain(ch, po):
                nc.vector.tensor_scalar_max(h1T_sb[:, ch, :], po[:], 0.0)

            out_phase(s1_sts, gath["s1"], relu_drain, "o1", load_wt=True)

            # ================= layer 2 =================
            t2_sts = t_phase(h1T_sb, C // 128, w2_sb, halves["t2"], "t2")
            all_gather(halves["t2"][0], gath["t2"][0])
            all_gather(halves["t2"][1], gath["t2"][1])
            s2_sts = s_phase(t2_sts, gath["t2"], f2_sb, halves["s2"], "s2")
            all_gather(halves["s2"][0], gath["s2"][0])
            all_gather(halves["s2"][1], gath["s2"][1])

            outst = outstp.tile([128, 2, R], F32)

            def out_drain(ch, po):
                nc.vector.tensor_copy(outst[:, ch, :], po[:])
                nc.sync.dma_start(
                    out=outT.ap()[ch * 128:(ch + 1) * 128, :], in_=outst[:, ch, :]
                )

            out_phase(s2_sts, gath["s2"], out_drain, "o2", load_wt=False)

    nc.compile()
    return nc


_NC_CACHE = {}


def _get_nc():
    if "nc" not in _NC_CACHE:
        _NC_CACHE["nc"] = build_kernel()
    return _NC_CACHE["nc"]


def _perm_winv(i):
    """winvT stream order for core i: local 1024 rows, then remote half-A
    (cyclic rank order), then remote half-B."""
    idx = [np.arange(i * R, (i + 1) * R)]
    for h in range(2):
        for s in range(NREM):
            rr = (i + 1 + s) % NCORES
            base = rr * R + h * HR
            idx.append(np.arange(base, base + HR))
    return np.concatenate(idx)


def _perm_wt(i):
    """wT program positions for core i: remote (h-major, cyclic rank order)
    at positions 0..55, local at 56..63."""
    idx = []
    for h in range(2):
        for s in range(NREM):
            rr = (i + 1 + s) % NCORES
            base = rr * R + h * HR
            idx.append(np.arange(base, base + HR))
    for h in range(2):
        base = i * R + h * HR
        idx.append(np.arange(base, base + HR))
    return np.concatenate(idx)


def make_in_maps(input, wavelets, wavelets_inv, W1, W2, filter1, filter2):
    input = np.asarray(input, np.float32)
    wavelets = np.asarray(wavelets, np.float32)
    wavelets_inv = np.asarray(wavelets_inv, np.float32)
    W1b = np.ascontiguousarray(np.asarray(W1, np.float32)).astype(NP_BF16)
    W2b = np.ascontiguousarray(np.asarray(W2, np.float32)).astype(NP_BF16)
    filter1 = np.asarray(filter1, np.float32)
    filter2 = np.asarray(filter2, np.float32)

    xTf = np.ascontiguousarray(input.T).astype(NP_BF16)   # [F, N]
    in_maps = []
    for i in range(NCORES):
        r0, r1 = i * R, (i + 1) * R
        winvT_i = np.ascontiguousarray(wavelets_inv[r0:r1].T).astype(NP_BF16)
        wT_i = np.ascontiguousarray(wavelets[r0:r1].T).astype(NP_BF16)
        in_maps.append(
            {
                "xT": np.ascontiguousarray(xTf[:, r0:r1]),
                "w1": W1b,
                "w2": W2b,
                "winvT": np.ascontiguousarray(winvT_i[_perm_winv(i)]),
                "wT": np.ascontiguousarray(wT_i[_perm_wt(i)]),
                "f1": np.ascontiguousarray(filter1[r0:r1]),
                "f2": np.ascontiguousarray(filter2[r0:r1]),
            }
        )
    return in_maps


def run(in_maps, trace=False, **kw):
    nc = _get_nc()
    return bass_utils.run_bass_kernel_spmd(
        nc, in_maps, core_ids=list(range(NCORES)), trace=trace, **kw
    )


def kernel(input, wavelets, wavelets_inv, W1, W2, filter1, filter2):
    in_maps = make_in_maps(
        input, wavelets, wavelets_inv, W1, W2, filter1, filter2
    )
    res = run(in_maps)
    out = np.empty((N, C), np.float32)
    for i in range(NCORES):
        out[i * R:(i + 1) * R, :] = res.results[i]["outT"].T
    return out


# revision 6
# speedup vs baseline: 4.3893x; 4.3893x over previous
"""Graph Wavelet NN (2-layer) Trainium2 kernel, 8-core row-parallel.

Math per layer: out = (wavelets * f) @ (wavelets_inv @ (x @ W)); the filter is
folded into a row-scale of the small spectral tensor s, so ONE bf16 copy of
wavelets[rows].T serves both layers and stays SBUF-resident (16MB/core).

Sharding: s = Winv @ t is computed row-sharded with full contraction per core;
the [rows_i, C] blocks of t and s are exchanged with half-split AllGathers
(256KB/rank each) pipelined against compute. The wavelet matrices are
host-permuted per core so the core's OWN contraction chunks sit at fixed
program positions, consumable from SBUF staging while the AllGather of the
remote chunks is still in flight (local head-start). Remote gather chunks are
loaded with partition-id-derived cyclic offsets ((pid+1+s) & 7) as dynamic-
offset DMAs on the otherwise idle scalar-engine queue, keeping the program
SPMD-uniform. Streaming winv loads live on the sync queue; collectives alone
on the gpsimd queue (their completion-waits would block anything queued
behind them). All heavy matmuls bf16 with fp32 PSUM accumulation; where two
256-col accumulators share a PSUM bank, only the first sets start=True
(start clears the whole bank).

Verified on HW: rel err 6.348e-3.
"""

import sys

if "/opt/trn_rl_repo" not in sys.path:
    sys.path.insert(0, "/opt/trn_rl_repo")

import numpy as np
import ml_dtypes

import concourse.bass as bass
import concourse.mybir as mybir
import concourse.tile as tile
from concourse import bacc, bass_utils

N = 8192
F = 512
C = 256
NCORES = 8
R = N // NCORES          # 1024 rows per core

F32 = mybir.dt.float32
BF16 = mybir.dt.bfloat16
NP_BF16 = ml_dtypes.bfloat16

NKC = N // 128           # 64 contraction chunks (program positions)
WVG = 4                  # kc chunks per 512-row stream tile
NWV = 16                 # stream tiles per layer
NMT = R // 128           # 8 local row tiles
HR = R // 2              # half row-block (512)
NREM = NCORES - 1        # 7 remote ranks


def build_kernel(sim_single_core=False):
    nc = bacc.Bacc(
        "TRN2",
        target_bir_lowering=False,
        debug=False,
        num_devices=1 if sim_single_core else NCORES,
    )

    xT = nc.dram_tensor("xT", [F, R], BF16, kind="ExternalInput")
    w1 = nc.dram_tensor("w1", [F, C], BF16, kind="ExternalInput")
    w2 = nc.dram_tensor("w2", [C, C], BF16, kind="ExternalInput")
    winvT = nc.dram_tensor("winvT", [N, R], BF16, kind="ExternalInput")
    wT = nc.dram_tensor("wT", [N, R], BF16, kind="ExternalInput")
    f1 = nc.dram_tensor("f1", [R], F32, kind="ExternalInput")
    f2 = nc.dram_tensor("f2", [R], F32, kind="ExternalInput")
    outT = nc.dram_tensor("outT", [C, R], F32, kind="ExternalOutput")

    rg = [list(range(NCORES))]

    with tile.TileContext(nc) as tc:
        with (
            tc.tile_pool(name="dram", bufs=1, space="DRAM") as dram,
            tc.tile_pool(name="const", bufs=1) as const,
            tc.tile_pool(name="winvp", bufs=4) as winvp,
            tc.tile_pool(name="tsp", bufs=6) as tsp,
            tc.tile_pool(name="stage", bufs=3) as stage,
            tc.tile_pool(name="outst", bufs=1) as outstp,
            tc.tile_pool(name="psA", bufs=2, space="PSUM") as psA,
            tc.tile_pool(name="psB", bufs=2, space="PSUM") as psB,
        ):
            halves = {}
            gath = {}
            for nm in ("t1", "s1", "t2", "s2"):
                halves[nm] = [
                    dram.tile([HR, C], BF16, name=f"{nm}{h}_d") for h in ("A", "B")
                ]
                gath[nm] = [
                    dram.tile(
                        [NCORES * HR, C], BF16,
                        addr_space="Local" if sim_single_core else "Shared",
                        name=f"{nm}{h}g_d",
                    )
                    for h in ("A", "B")
                ]

            # ---- persistent SBUF ----
            wT_sb = const.tile([128, NKC, R], BF16)        # 128KB/part, resident
            xT_sb = const.tile([128, F // 128, R], BF16)   # 8KB/part
            w1_sb = const.tile([128, F // 128, C], BF16)   # 2KB/part
            w2_sb = const.tile([128, C // 128, C], BF16)   # 1KB/part
            h1T_sb = const.tile([128, C // 128, R], BF16)  # 4KB/part
            f1_sb = const.tile([128, NMT], F32)
            f2_sb = const.tile([128, NMT], F32)

            nc.sync.dma_start(
                out=w1_sb[:, 0:2, :],
                in_=w1.ap()[0:256, :].rearrange("(kc p) n -> p kc n", p=128),
            )
            for q in range(2):
                nc.sync.dma_start(
                    out=xT_sb[:, q * 2:(q + 1) * 2, :],
                    in_=xT.ap()[q * 256:(q + 1) * 256, :].rearrange(
                        "(kc p) m -> p kc m", p=128
                    ),
                )
            nc.sync.dma_start(
                out=w1_sb[:, 2:4, :],
                in_=w1.ap()[256:512, :].rearrange("(kc p) n -> p kc n", p=128),
            )
            nc.sync.dma_start(
                out=w2_sb[:], in_=w2.ap().rearrange("(kc p) n -> p kc n", p=128)
            )
            nc.sync.dma_start(
                out=f1_sb[:], in_=f1.ap().rearrange("(mt p) -> p mt", p=128)
            )
            nc.sync.dma_start(
                out=f2_sb[:], in_=f2.ap().rearrange("(mt p) -> p mt", p=128)
            )

            def all_gather(in_d, out_d):
                if sim_single_core:
                    for rr in range(NCORES):
                        nc.sync.dma_start(
                            out=out_d[rr * HR:(rr + 1) * HR, :], in_=in_d[:, :]
                        )
                else:
                    nc.gpsimd.collective_compute(
                        "AllGather",
                        mybir.AluOpType.bypass,
                        replica_groups=rg,
                        ins=[in_d.opt()],
                        outs=[out_d.opt()],
                    )

            # cyclic remote-rank offsets; host permutes wavelet rows to match
            pid = nc.scalar.partition_id()
            rrs = [(pid + (1 + s)) & 7 for s in range(NREM)]

            # wT program-position chunk-groups are contiguous rows of the
            # host-permuted tensor: group g = positions 4g..4g+3.
            def wt_load(g, eng=None):
                (eng or nc.sync).dma_start(
                    out=wT_sb[:, g * WVG:(g + 1) * WVG, :],
                    in_=wT.ap()[g * 512:(g + 1) * 512, :].rearrange(
                        "(kc p) m -> p kc m", p=128
                    ),
                )

            # ---- t = x_loc @ W, halves staged + stored + gathered ----
            def t_phase(lhsT_sb, nkc, w_sb, t_ds, name):
                sts = []
                for h in range(2):
                    st = stage.tile([128, 4, C], BF16, tag="st", name=f"st_{name}{h}")
                    sts.append(st)
                    for m4 in range(4):
                        mt = h * 4 + m4
                        pt = psA.tile(
                            [128, 2, 2, C], F32, tag="psA", name=f"pt_{name}{mt}"
                        )
                        for kc in range(nkc):
                            nc.tensor.matmul(
                                pt[:, 0, 0, :],
                                lhsT_sb[:, kc, mt * 128:(mt + 1) * 128],
                                w_sb[:, kc, :],
                                start=(kc == 0),
                                stop=(kc == nkc - 1),
                            )
                        nc.vector.tensor_copy(st[:, m4, :], pt[:, 0, 0, :])
                    nc.sync.dma_start(
                        out=t_ds[h][:, :].rearrange("(mt p) n -> p mt n", p=128),
                        in_=st[:],
                    )
                return sts

            # ---- s_loc = Winv[rows,:] @ t_full (permuted contraction order:
            # local rows first from staging, then remote half-A, then half-B),
            # scaled by filter[rows] ----
            def s_phase(t_sts, tg_ds, f_sb, s_ds, name):
                ps = [
                    psA.tile([128, 2, 2, C], F32, tag="psA", name=f"ps_{name}{i}")
                    for i in range(2)
                ]
                # rhs chunk supplier per stream tile index (16 total)
                rhs_of = {}
                rhs_of[0] = lambda k4: t_sts[0][:, k4, :]
                rhs_of[1] = lambda k4: t_sts[1][:, k4, :]
                for h in range(2):
                    for s in range(NREM):
                        tch = tsp.tile(
                            [128, 4, C], BF16, tag="ts", name=f"tch_{name}{h}{s}"
                        )
                        nc.scalar.dma_start(
                            out=tch[:],
                            in_=tg_ds[h][bass.ts(rrs[s], HR), :].rearrange(
                                "(kc p) n -> p kc n", p=128
                            ),
                        )
                        rhs_of[2 + h * NREM + s] = (
                            lambda k4, _t=tch: _t[:, k4, :]
                        )
                for wv_i in range(NWV):
                    wv = winvp.tile(
                        [128, WVG, R], BF16, tag="wv", name=f"wv_{name}{wv_i}"
                    )
                    nc.sync.dma_start(
                        out=wv[:],
                        in_=winvT.ap()[wv_i * 512:(wv_i + 1) * 512, :].rearrange(
                            "(kc p) m -> p kc m", p=128
                        ),
                    )
                    for k4 in range(WVG):
                        for mt in range(NMT):
                            # two 256-col accumulators share each PSUM bank;
                            # only the first of each pair may set start=True
                            # (start clears the whole bank)
                            nc.tensor.matmul(
                                ps[mt // 4][:, (mt % 4) // 2, (mt % 4) % 2, :],
                                wv[:, k4, mt * 128:(mt + 1) * 128],
                                rhs_of[wv_i](k4),
                                start=(wv_i == 0 and k4 == 0 and mt % 2 == 0),
                                stop=(wv_i == NWV - 1 and k4 == WVG - 1),
                                skip_group_check=True,
                            )
                sts = []
                for h in range(2):
                    st = stage.tile([128, 4, C], BF16, tag="st", name=f"sst_{name}{h}")
                    sts.append(st)
                    for m4 in range(4):
                        mt = h * 4 + m4
                        nc.vector.tensor_scalar_mul(
                            st[:, m4, :],
                            ps[mt // 4][:, (mt % 4) // 2, (mt % 4) % 2, :],
                            f_sb[:, mt:mt + 1],
                        )
                    nc.sync.dma_start(
                        out=s_ds[h][:, :].rearrange("(mt p) n -> p mt n", p=128),
                        in_=st[:],
                    )
                return sts

            # ---- outT = (wavelets[rows] @ s_full).T, local positions first ----
            def out_phase(s_sts, sg_ds, drain_cb, name, load_wt):
                po = [
                    psB.tile([128, R], F32, tag="psB", name=f"po_{name}{i}")
                    for i in range(2)
                ]
                n_kc = 0

                def mm4(pos, lhsT_of):
                    nonlocal n_kc
                    for k4 in range(WVG):
                        kc = pos + k4
                        for ch in range(2):
                            for mh in range(2):
                                nc.tensor.matmul(
                                    po[ch][:, mh * 512:(mh + 1) * 512],
                                    lhsT_of(k4, ch),
                                    wT_sb[:, kc, mh * 512:(mh + 1) * 512],
                                    start=(n_kc == 0),
                                    stop=(n_kc == NKC - 1),
                                    skip_group_check=True,
                                )
                        n_kc += 1

                # local head-start: positions 56..63 from s staging
                for h in range(2):
                    st = s_sts[h]
                    mm4(
                        56 + h * 4,
                        lambda k4, ch, _st=st: _st[:, k4, ch * 128:(ch + 1) * 128],
                    )
                # remote sweep: positions h*28 + s*4
                wt_g = 4
                for h in range(2):
                    for s in range(NREM):
                        sch = tsp.tile(
                            [128, 4, C], BF16, tag="ts", name=f"sch_{name}{h}{s}"
                        )
                        nc.scalar.dma_start(
                            out=sch[:],
                            in_=sg_ds[h][bass.ts(rrs[s], HR), :].rearrange(
                                "(kc p) n -> p kc n", p=128
                            ),
                        )
                        if load_wt:
                            for _ in range(2):
                                if wt_g < 14:
                                    wt_load(wt_g)
                                    wt_g += 1
                        mm4(
                            h * 28 + s * 4,
                            lambda k4, ch, _t=sch: _t[:, k4, ch * 128:(ch + 1) * 128],
                        )
                for ch in range(2):
                    drain_cb(ch, po[ch])

            # ================= layer 1 =================
            # upfront wT: local positions (56-63) + first remote groups (0-7);
            # on the scalar queue so they don't delay the winv stream on SP
            for g in (14, 15, 0, 1):
                wt_load(g, eng=nc.scalar)

            t1_sts = t_phase(xT_sb, F // 128, w1_sb, halves["t1"], "t1")
            all_gather(halves["t1"][0], gath["t1"][0])
            all_gather(halves["t1"][1], gath["t1"][1])
            s1_sts = s_phase(t1_sts, gath["t1"], f1_sb, halves["s1"], "s1")
            wt_load(2)
            wt_load(3)
            all_gather(halves["s1"][0], gath["s1"][0])
            all_gather(halves["s1"][1], gath["s1"][1])

            def relu_drain(ch, po):
                # m-halves: first halves unblock t2's A-half matmuls early
                for mh in range(2):
                    nc.vector.tensor_scalar_max(
                        h1T_sb[:, ch, mh * 512:(mh + 1) * 512],
                        po[:, mh * 512:(mh + 1) * 512],
                        0.0,
                    )

            out_phase(s1_sts, gath["s1"], relu_drain, "o1", load_wt=True)

            # ================= layer 2 =================
            t2_sts = t_phase(h1T_sb, C // 128, w2_sb, halves["t2"], "t2")
            all_gather(halves["t2"][0], gath["t2"][0])
            all_gather(halves["t2"][1], gath["t2"][1])
            s2_sts = s_phase(t2_sts, gath["t2"], f2_sb, halves["s2"], "s2")
            all_gather(halves["s2"][0], gath["s2"][0])
            all_gather(halves["s2"][1], gath["s2"][1])

            outst = outstp.tile([128, 2, R], F32)

            def out_drain(ch, po):
                for mh in range(2):
                    nc.vector.tensor_copy(
                        outst[:, ch, mh * 512:(mh + 1) * 512],
                        po[:, mh * 512:(mh + 1) * 512],
                    )
                    nc.sync.dma_start(
                        out=outT.ap()[
                            ch * 128:(ch + 1) * 128, mh * 512:(mh + 1) * 512
                        ],
                        in_=outst[:, ch, mh * 512:(mh + 1) * 512],
                    )

            out_phase(s2_sts, gath["s2"], out_drain, "o2", load_wt=False)

    nc.compile()
    return nc


_NC_CACHE = {}


def _get_nc():
    if "nc" not in _NC_CACHE:
        _NC_CACHE["nc"] = build_kernel()
    return _NC_CACHE["nc"]


def _perm_winv(i):
    """winvT stream order for core i: local 1024 rows, then remote half-A
    (cyclic rank order), then remote half-B."""
    idx = [np.arange(i * R, (i + 1) * R)]
    for h in range(2):
        for s in range(NREM):
            rr = (i + 1 + s) % NCORES
            base = rr * R + h * HR
            idx.append(np.arange(base, base + HR))
    return np.concatenate(idx)


def _perm_wt(i):
    """wT program positions for core i: remote (h-major, cyclic rank order)
    at positions 0..55, local at 56..63."""
    idx = []
    for h in range(2):
        for s in range(NREM):
            rr = (i + 1 + s) % NCORES
            base = rr * R + h * HR
            idx.append(np.arange(base, base + HR))
    for h in range(2):
        base = i * R + h * HR
        idx.append(np.arange(base, base + HR))
    return np.concatenate(idx)


def make_in_maps(input, wavelets, wavelets_inv, W1, W2, filter1, filter2):
    input = np.asarray(input, np.float32)
    wavelets = np.asarray(wavelets, np.float32)
    wavelets_inv = np.asarray(wavelets_inv, np.float32)
    W1b = np.ascontiguousarray(np.asarray(W1, np.float32)).astype(NP_BF16)
    W2b = np.ascontiguousarray(np.asarray(W2, np.float32)).astype(NP_BF16)
    filter1 = np.asarray(filter1, np.float32)
    filter2 = np.asarray(filter2, np.float32)

    xTf = np.ascontiguousarray(input.T).astype(NP_BF16)   # [F, N]
    in_maps = []
    for i in range(NCORES):
        r0, r1 = i * R, (i + 1) * R
        winvT_i = np.ascontiguousarray(wavelets_inv[r0:r1].T).astype(NP_BF16)
        wT_i = np.ascontiguousarray(wavelets[r0:r1].T).astype(NP_BF16)
        in_maps.append(
            {
                "xT": np.ascontiguousarray(xTf[:, r0:r1]),
                "w1": W1b,
                "w2": W2b,
                "winvT": np.ascontiguousarray(winvT_i[_perm_winv(i)]),
                "wT": np.ascontiguousarray(wT_i[_perm_wt(i)]),
                "f1": np.ascontiguousarray(filter1[r0:r1]),
                "f2": np.ascontiguousarray(filter2[r0:r1]),
            }
        )
    return in_maps


def run(in_maps, trace=False, **kw):
    nc = _get_nc()
    return bass_utils.run_bass_kernel_spmd(
        nc, in_maps, core_ids=list(range(NCORES)), trace=trace, **kw
    )


def kernel(input, wavelets, wavelets_inv, W1, W2, filter1, filter2):
    in_maps = make_in_maps(
        input, wavelets, wavelets_inv, W1, W2, filter1, filter2
    )
    res = run(in_maps)
    out = np.empty((N, C), np.float32)
    for i in range(NCORES):
        out[i * R:(i + 1) * R, :] = res.results[i]["outT"].T
    return out
